# revision 34
# baseline (speedup 1.0000x reference)
"""GAT (3-layer, PyG-style) Trainium2 Bass kernel, 8-core dst-sharded. v2.

Self-contained: takes full inputs, shards internally, returns full output.

Design (per layer -> two SPMD launches):
  Launch A (dense, src-sharded): each core computes its 1/8 of the gather
    table rows [64 feat bf16 | 4 al_src f32] = 256B/row via PE matmuls with
    Waug = [W(*bns) | W@blk(a_src) | W@blk(a_dst)], plus per-dst al_dst and
    the dense skip+bias rows for its dst shard. Host assembles the full
    table (plus 2 sentinel rows) and expands the per-slot al_dst stream.
  Launch B (edge, dst-sharded): exact-CSR edge slots (chunks of 128 slots,
    one (window, src-half) per chunk; schedule shared across cores, per-core
    counts balanced by an LPT assignment of dst nodes to (core, window)).
    Per group of chunks: dma_gather of 256B rows (4 SWDGE queues round-
    robin), ex = exp(leaky_relu(al_s + al_d)) on DVE/Act, messages scaled
    by ex, segment-sum via per-chunk one-hot matmuls (sel built on DVE by
    comparing a shipped dstloc column against an iota row) accumulating
    [msg | sum_ex] per 128-dst window in PSUM, drained into an SBUF
    accumulator. One batched normalize pass at the end.
  Layer 2 uses GAT linearity: segment-sum runs in h1-space (same 256B rows)
    with 4 per-head ex-scaled copies; W2 is applied per window afterwards
    (PE transpose + 2 matmuls), then mean over heads.
"""
import numpy as np
import ml_dtypes

import concourse.bacc as bacc
import concourse.mybir as mybir
import concourse.tile as tile
from concourse.alu_op_type import AluOpType
from concourse.bass_utils import run_bass_kernel_spmd

BF16 = mybir.dt.bfloat16
F32 = mybir.dt.float32
I16 = mybir.dt.int16

NC = 8
P = 128
EPS = 1e-5
SENT_ALS = -40.0
NQ = 4          # SWDGE queues
GC01 = 68       # chunks per gather group, layers 0/1
GC2 = 48        # chunks per gather group, layer 2

N_NODES = 50000
SB = N_NODES // NC          # src nodes per core (6250)
NWIN = 51                   # dst windows per core
WCAP = 125                  # dsts per window (< 128 for ceil slack)
DP = NWIN * P               # padded dst slots per core (6528)
RB = DP                     # table row block per core (6272)
LO_ROWS = 4 * RB            # rows in the lo half (25088)
SENT_REL = LO_ROWS          # sentinel index relative to each half base
TROWS = 2 * (LO_ROWS + 1)


def _ceil(a, b):
    return -(-a // b)


# ----------------------------------------------------------------- planning

def build_plan(src, dst, N):
    assert N == N_NODES
    # src -> permuted table row (relative to half base) and half flag
    s_core = src // SB
    arow = (src % SB) + s_core * RB
    half = (s_core >= 4).astype(np.int64)
    rel = np.where(half == 0, arow, arow - 4 * RB)

    # balanced assignment of dst nodes to (core, window) buckets, keeping
    # BOTH halves' per-bucket loads even (ceil(max_c/128) drives slot count)
    deg = np.zeros((N, 2), np.int64)
    np.add.at(deg, (dst, half), 1)
    tot = deg.sum(1)
    order = np.argsort(-tot, kind="stable")
    NB = NC * NWIN
    LO = np.zeros(NB, np.float64)
    HI = np.zeros(NB, np.float64)
    bcount = np.zeros(NB, np.int64)
    BIG = 1e18
    dst_c = np.empty(N, np.int64)
    dst_w = np.empty(N, np.int64)
    dst_p = np.empty(N, np.int64)
    for n in order:
        score = np.maximum(LO + deg[n, 0], HI + deg[n, 1])
        b = int(np.argmin(score))
        dst_c[n] = b // NWIN
        dst_w[n] = b % NWIN
        dst_p[n] = bcount[b]
        bcount[b] += 1
        LO[b] += deg[n, 0]
        HI[b] += deg[n, 1]
        if bcount[b] >= WCAP:
            LO[b] = HI[b] = BIG

    vmap = np.full((NC, DP), -1, np.int64)
    vmap[dst_c, dst_w * P + dst_p] = np.arange(N)

    # per (core, window, half) edge counts -> shared chunk schedule
    ec = dst_c[dst]
    ew = dst_w[dst]
    epos = dst_p[dst]
    cnt = np.zeros((NC, NWIN, 2), np.int64)
    np.add.at(cnt, (ec, ew, half), 1)
    nch = np.zeros((NWIN, 2), np.int64)
    for w in range(NWIN):
        for h in range(2):
            nch[w, h] = _ceil(int(cnt[:, w, h].max()), P) if cnt[:, w, h].max() else 0
        if nch[w].sum() == 0:
            nch[w, 0] = 1  # keep every window on the schedule
    chunks = []          # (w, h) in schedule order: lo sweep then hi sweep
    for h in range(2):
        for w in range(NWIN):
            chunks += [(w, h)] * int(nch[w, h])
    nchunks = len(chunks)
    slot_base = {}       # (w, h) -> first slot index in the combined stream
    o = 0
    for (w, h) in chunks:
        slot_base.setdefault((w, h), o * P)
        o += 1
    nlo = int(nch[:, 0].sum())
    SLO, SHI = nlo * P, (nchunks - nlo) * P

    # per-core streams
    plans = []
    for c in range(NC):
        em = ec == c
        eh = half[em]
        key = eh * (NWIN * P) + ew[em] * P  # per (h, w) group key base
        # stable ordering by (h, w); position within group via argsort
        si = np.argsort(key, kind="stable")
        erel = rel[em][si]
        ewk = ew[em][si]
        ehk = eh[si]
        epk = epos[em][si]
        idx_all = np.full(nchunks * P, SENT_REL, np.int64)
        dloc = np.full(nchunks * P, -1.0, np.float32)
        sdst = np.zeros(nchunks * P, np.int64)
        # group runs: edges sorted by (h, w); fill each group's slot range
        bounds = np.searchsorted(
            ehk * NWIN + ewk, np.arange(2 * NWIN + 1))
        for h in range(2):
            for w in range(NWIN):
                a, b = bounds[h * NWIN + w], bounds[h * NWIN + w + 1]
                if a == b:
                    continue
                s0 = slot_base[(w, h)]
                m = b - a
                idx_all[s0: s0 + m] = erel[a:b]
                dloc[s0: s0 + m] = epk[a:b]
                sdst[s0: s0 + m] = w * P + epk[a:b]
        idx_lo = _wrap16(idx_all[:SLO])
        idx_hi = _wrap16(idx_all[SLO:])
        dstloc = dloc.reshape(nchunks, P).T.astype(ml_dtypes.bfloat16)
        sdst_w = sdst.reshape(nchunks, P).transpose(1, 0)
        plans.append(dict(vmap=vmap[c], idx_lo=idx_lo, idx_hi=idx_hi,
                          dstloc=np.ascontiguousarray(dstloc),
                          slot_dst=np.ascontiguousarray(sdst_w)))
    shared = dict(chunks=chunks, nchunks=nchunks, nlo=nlo, SLO=SLO, SHI=SHI)
    return shared, plans


def _wrap16(stream):
    S = len(stream)
    w = stream.reshape(S // 16, 16).T.astype(np.int16)
    return np.ascontiguousarray(np.tile(w, (8, 1)))


PERMC = (np.arange(DP) % P) * NWIN + np.arange(DP) // P  # col k -> slot p*49+i


# ------------------------------------------------------------- launch A (dense)

def build_dense(F):
    nc = bacc.Bacc("TRN2", target_bir_lowering=False, debug=False)
    hTs = nc.dram_tensor("hTs", [F, DP], BF16, kind="ExternalInput")
    hTow = nc.dram_tensor("hTow", [F, DP], BF16, kind="ExternalInput")
    Waug = nc.dram_tensor("Waug", [F, 72], BF16, kind="ExternalInput")
    skipW = nc.dram_tensor("skipW", [F, 64], BF16, kind="ExternalInput")
    biasR = nc.dram_tensor("biasR", [P, 64], F32, kind="ExternalInput")
    tshard = nc.dram_tensor("tshard", [DP, P], BF16, kind="ExternalOutput")
    aldv = nc.dram_tensor("aldv", [DP, 4], F32, kind="ExternalOutput")
    skipd = nc.dram_tensor("skipd", [DP, 64], F32, kind="ExternalOutput")
    featd = nc.dram_tensor("featd", [DP, 64], BF16, kind="ExternalOutput")
    selfz = nc.dram_tensor("selfz", [DP, 4], F32, kind="ExternalOutput")

    with tile.TileContext(nc) as tc:
        with (
            tc.tile_pool(name="c", bufs=1) as cp,
            tc.tile_pool(name="ps", bufs=2, space="PSUM") as pp,
        ):
            hts_sb = cp.tile([F, DP], BF16)
            nc.sync.dma_start(hts_sb[:], hTs[:])
            htow_sb = cp.tile([F, DP], BF16)
            nc.sync.dma_start(htow_sb[:], hTow[:])
            waug_sb = cp.tile([F, 72], BF16)
            nc.sync.dma_start(waug_sb[:], Waug[:])
            skipw_sb = cp.tile([F, 64], BF16)
            nc.sync.dma_start(skipw_sb[:], skipW[:])
            bias_sb = cp.tile([P, 64], F32)
            nc.sync.dma_start(bias_sb[:], biasR[:])

            tstage = cp.tile([P, NWIN, P], BF16)
            asb = cp.tile([P, NWIN, 4], F32)
            ssb = cp.tile([P, NWIN, 64], F32)
            fsb = cp.tile([P, NWIN, 64], BF16)
            szb = cp.tile([P, NWIN, 4], F32)
            nc.gpsimd.memset(tstage[:], 0)
            for i in range(NWIN):
                dps = pp.tile([P, 72], F32, space="PSUM", tag="dps")
                nc.tensor.matmul(dps[:], hts_sb[:, i * P:(i + 1) * P],
                                 waug_sb[:], start=True, stop=True)
                nc.vector.tensor_copy(tstage[:, i, 0:64], dps[:, 0:64])
                tf32 = tstage[:].bitcast(F32)
                nc.vector.tensor_copy(tf32[:, i, 32:36], dps[:, 64:68])
                ops_ = pp.tile([P, 72], F32, space="PSUM", tag="ops")
                nc.tensor.matmul(ops_[:], htow_sb[:, i * P:(i + 1) * P],
                                 waug_sb[:], start=True, stop=True)
                sps = pp.tile([P, 64], F32, space="PSUM", tag="sps")
                nc.tensor.matmul(sps[:], htow_sb[:, i * P:(i + 1) * P],
                                 skipw_sb[:], start=True, stop=True)
                nc.vector.tensor_copy(asb[:, i, :], ops_[:, 68:72])
                nc.vector.tensor_copy(fsb[:, i, :], ops_[:, 0:64])
                nc.vector.tensor_tensor(szb[:, i, :], ops_[:, 64:68],
                                        asb[:, i, :], AluOpType.add)
                nc.vector.tensor_tensor(ssb[:, i, :], sps[:],
                                        bias_sb[:], AluOpType.add)
            nc.sync.dma_start(
                tshard[:].rearrange("(p i) w -> p i w", p=P), tstage[:])
            nc.sync.dma_start(
                aldv[:].rearrange("(p i) h -> p i h", p=P), asb[:])
            nc.sync.dma_start(
                skipd[:].rearrange("(p i) c -> p i c", p=P), ssb[:])
            nc.sync.dma_start(
                featd[:].rearrange("(p i) c -> p i c", p=P), fsb[:])
            nc.sync.dma_start(
                selfz[:].rearrange("(p i) h -> p i h", p=P), szb[:])
    nc.compile()
    return nc


# ------------------------------------------------------------- launch B (edge)

def build_edge(shared, l2):
    chunks = shared["chunks"]
    nchunks = shared["nchunks"]
    nlo = shared["nlo"]
    SLO, SHI = shared["SLO"], shared["SHI"]
    GC = GC2 if l2 else GC01
    NW = 260 if l2 else 68

    nc = bacc.Bacc("TRN2", target_bir_lowering=False, debug=False,
                   num_swdge_queues=NQ)
    table = nc.dram_tensor("table", [TROWS, P], BF16, kind="ExternalInput")
    idx_lo = nc.dram_tensor("idx_lo", [P, max(SLO, 16) // 16], I16,
                            kind="ExternalInput")
    idx_hi = nc.dram_tensor("idx_hi", [P, max(SHI, 16) // 16], I16,
                            kind="ExternalInput")
    dstloc = nc.dram_tensor("dstloc", [P, nchunks], BF16,
                            kind="ExternalInput")
    alde_in = nc.dram_tensor("alde", [P, nchunks, 4], F32,
                             kind="ExternalInput")
    skipd_in = nc.dram_tensor("skipd", [DP, 64], F32, kind="ExternalInput")
    featd_in = nc.dram_tensor("featd", [DP, 64], BF16, kind="ExternalInput")
    selfz_in = nc.dram_tensor("selfz", [DP, 4], F32, kind="ExternalInput")
    iota_in = nc.dram_tensor("iota", [P, P], BF16, kind="ExternalInput")
    if l2:
        w2_in = nc.dram_tensor("w2", [P, 2, 64], BF16, kind="ExternalInput")
        ident_in = nc.dram_tensor("ident", [P, P], BF16, kind="ExternalInput")
    y_out = nc.dram_tensor("y", [DP, 64], F32, kind="ExternalOutput")

    # group schedule: runs of <= GC chunks, same half
    groups = []
    k = 0
    while k < nchunks:
        k1 = min(k + GC, nchunks, nlo if k < nlo else nchunks)
        groups.append((k, k1))
        k = k1
    first = [False] * nchunks
    last = [False] * nchunks
    wlast = [False] * nchunks
    seen = set()
    wl = {}
    for k, (w, h) in enumerate(chunks):
        if (h, w) not in seen:
            seen.add((h, w))
            first[k] = True
        if k + 1 >= nchunks or chunks[k + 1] != (w, h):
            last[k] = True
        wl[w] = k
    for w, k in wl.items():
        wlast[k] = True

    with tile.TileContext(nc) as tc:
        with (
            tc.tile_pool(name="c", bufs=1) as cp,
            tc.tile_pool(name="g", bufs=3 if l2 else 4) as gp,
            tc.tile_pool(name="r", bufs=2) as rp,
            tc.tile_pool(name="s", bufs=3) as sp,
            tc.tile_pool(name="pw", bufs=4, space="PSUM") as pw,
            tc.tile_pool(name="pt", bufs=2, space="PSUM") as pt,
        ):
            # big constant loads go on the Act engine's DGE so the first
            # gather groups' idx loads (sync engine) aren't queued behind them
            dloc_sb = cp.tile([P, nchunks], BF16)
            nc.scalar.dma_start(dloc_sb[:], dstloc[:])
            if not l2:
                alde_full = cp.tile([P, nchunks, 4], F32)
                nc.scalar.dma_start(alde_full[:], alde_in[:])
            skipd_sb = cp.tile([P, NWIN, 64], F32)
            nc.scalar.dma_start(
                skipd_sb[:], skipd_in[:].rearrange("(i p) c -> p i c", p=P))
            featd_sb = cp.tile([P, NWIN, 64], BF16)
            nc.scalar.dma_start(
                featd_sb[:], featd_in[:].rearrange("(i p) c -> p i c", p=P))
            selfz_sb = cp.tile([P, NWIN, 4], F32)
            nc.scalar.dma_start(
                selfz_sb[:], selfz_in[:].rearrange("(i p) h -> p i h", p=P))
            iota_sb = cp.tile([P, 1, P], BF16)
            nc.scalar.dma_start(iota_sb[:, 0, :], iota_in[:])
            if l2:
                w2_sb = cp.tile([P, 2, 64], BF16)
                nc.scalar.dma_start(w2_sb[:], w2_in[:])
                ident_sb = cp.tile([P, P], BF16)
                nc.scalar.dma_start(ident_sb[:], ident_in[:])
            msum = cp.tile([P, NWIN, NW], F32)
            y_sb = cp.tile([P, NWIN, 64], F32)

            # init msum with the dense self-loop contributions
            exself = cp.tile([P, NWIN, 4], F32)
            nc.vector.scalar_tensor_tensor(
                exself[:], selfz_sb[:], 0.2, selfz_sb[:],
                AluOpType.mult, AluOpType.max)
            nc.scalar.activation(exself[:], exself[:],
                                 mybir.ActivationFunctionType.Exp)
            nc.vector.tensor_copy(msum[:, :, NW - 4: NW], exself[:])
            if l2:
                nc.vector.tensor_tensor(
                    msum[:, :, 0:256].rearrange("p w (h c) -> p w h c", c=64),
                    featd_sb[:, :, None, :].to_broadcast([P, NWIN, 4, 64]),
                    exself[:, :, :, None].to_broadcast([P, NWIN, 4, 64]),
                    AluOpType.mult)
            else:
                nc.vector.tensor_tensor(
                    msum[:, :, 0:64].rearrange("p w (h c) -> p w h c", c=16),
                    featd_sb[:].rearrange("p w (h c) -> p w h c", c=16),
                    exself[:, :, :, None].to_broadcast([P, NWIN, 4, 16]),
                    AluOpType.mult)

            win_ps = {}
            for gi, (k0, k1) in enumerate(groups):
                T = k1 - k0
                h = chunks[k0][1]
                base = table[0: LO_ROWS + 1, :] if h == 0 \
                    else table[LO_ROWS + 1: TROWS, :]
                o16 = (k0 * P if h == 0 else (k0 - nlo) * P) // 16
                idx_t = sp.tile([P, GC * 8], I16, tag="idx")
                nc.sync.dma_start(
                    idx_t[:, : T * 8],
                    (idx_lo if h == 0 else idx_hi)[:, o16: o16 + T * 8])
                gt = gp.tile([P, GC, P], BF16, tag="g")
                nc.gpsimd.dma_gather(
                    gt[:, :T, :], base, idx_t[:, : T * 8], T * P, T * P, P,
                    single_packet=False, queue_num=gi % NQ)

                if l2:
                    alde_t = sp.tile([P, GC, 4], F32, tag="alde")
                    nc.scalar.dma_start(alde_t[:, :T, :],
                                        alde_in[:, k0:k1, :])
                    alde_ap = alde_t[:, :T, :]
                else:
                    alde_ap = alde_full[:, k0:k1, :]
                zf = sp.tile([P, GC, 4], F32, tag="z")
                gf = gt[:].bitcast(F32)
                nc.vector.tensor_tensor(zf[:, :T, :], gf[:, :T, 32:36],
                                        alde_ap, AluOpType.add)
                nc.vector.scalar_tensor_tensor(
                    zf[:, :T, :], zf[:, :T, :], 0.2, zf[:, :T, :],
                    AluOpType.mult, AluOpType.max)
                sel = (rp if l2 else sp).tile([P, GC, P], BF16, tag="sel")
                nc.vector.tensor_tensor(
                    sel[:, :T, :],
                    iota_sb[:].to_broadcast([P, T, P]),
                    dloc_sb[:, k0:k1, None].to_broadcast([P, T, P]),
                    AluOpType.is_equal)

                if l2:
                    rhs = rp.tile([P, GC, 260], BF16, tag="rhs")
                    nc.scalar.activation(rhs[:, :T, 256:260], zf[:, :T, :],
                                         mybir.ActivationFunctionType.Exp)
                    nc.vector.tensor_tensor(
                        rhs[:, :T, 0:256].rearrange(
                            "p t (h c) -> p t h c", c=64),
                        gt[:, :T, None, 0:64].to_broadcast([P, T, 4, 64]),
                        rhs[:, :T, 256:260, None].to_broadcast([P, T, 4, 64]),
                        AluOpType.mult)
                else:
                    nc.scalar.activation(gt[:, :T, 64:68], zf[:, :T, :],
                                         mybir.ActivationFunctionType.Exp)
                    nc.vector.tensor_tensor(
                        gt[:, :T, 0:64].rearrange("p t (h c) -> p t h c", c=16),
                        gt[:, :T, 0:64].rearrange("p t (h c) -> p t h c", c=16),
                        gt[:, :T, 64:68, None].to_broadcast([P, T, 4, 16]),
                        AluOpType.mult)

                for t in range(T):
                    k = k0 + t
                    w, hh = chunks[k]
                    if first[k]:
                        win_ps[w] = pw.tile([P, NW], F32, space="PSUM",
                                            tag="win", name=f"win{w}h{hh}")
                    rhs_ap = rhs[:, t, :] if l2 else gt[:, t, 0:68]
                    nc.tensor.matmul(win_ps[w][:], sel[:, t, :], rhs_ap,
                                     start=first[k], stop=last[k],
                                     skip_group_check=True)
                    if last[k]:
                        pwin = win_ps.pop(w)
                        nc.vector.tensor_tensor(msum[:, w, :], msum[:, w, :],
                                                pwin[:], AluOpType.add)
                    if l2 and wlast[k]:
                        # per-window W2 drain, overlapped with later groups
                        recw = sp.tile([P, 4], F32, tag="recw")
                        nc.vector.reciprocal(recw[:], msum[:, w, 256:260])
                        snw = sp.tile([P, 4, 64], BF16, tag="snw")
                        nc.vector.tensor_tensor(
                            snw[:],
                            msum[:, w, 0:256].rearrange(
                                "p (h c) -> p h c", c=64),
                            recw[:, :, None].to_broadcast([P, 4, 64]),
                            AluOpType.mult)
                        yps = pt.tile([P, 64], F32, space="PSUM", tag="yps")
                        for j in range(2):
                            tp = pt.tile([P, P], BF16, space="PSUM", tag="tp")
                            nc.tensor.matmul(
                                tp[:],
                                snw[:].rearrange("p h c -> p (h c)")
                                      [:, j * P:(j + 1) * P],
                                ident_sb[:], is_transpose=True,
                                start=True, stop=True, skip_group_check=True)
                            st = sp.tile([P, P], BF16, tag="st")
                            nc.scalar.activation(
                                st[:], tp[:],
                                mybir.ActivationFunctionType.Copy)
                            nc.tensor.matmul(yps[:], st[:], w2_sb[:, j, :],
                                             start=(j == 0), stop=(j == 1),
                                             skip_group_check=True)
                        nc.vector.scalar_tensor_tensor(
                            y_sb[:, w, :], yps[:], 0.25, skipd_sb[:, w, :],
                            AluOpType.mult, AluOpType.add)

            if not l2:
                rec = cp.tile([P, NWIN, 4], F32)
                nc.vector.reciprocal(rec[:], msum[:, :, 64:68])
                nc.vector.tensor_tensor(
                    y_sb[:].rearrange("p w (h c) -> p w h c", c=16),
                    msum[:, :, 0:64].rearrange("p w (h c) -> p w h c", c=16),
                    rec[:, :, :, None].to_broadcast([P, NWIN, 4, 16]),
                    AluOpType.mult)
                nc.vector.tensor_tensor(y_sb[:], y_sb[:], skipd_sb[:],
                                        AluOpType.add)
                nc.vector.tensor_scalar_max(y_sb[:], y_sb[:], 0.0)
            nc.sync.dma_start(
                y_out[:].rearrange("(i p) c -> p i c", p=P), y_sb[:])
    nc.compile()
    return nc


# ------------------------------------------------------------------ driver

_CACHE = {}
_DBG = []
_EXEC_NS = []
_RESULTS = []


def _blockdiag(a):
    H, C = a.shape
    m = np.zeros((H * C, H), np.float32)
    for hh in range(H):
        m[hh * C: (hh + 1) * C, hh] = a[hh]
    return m


def _bf(x):
    return np.ascontiguousarray(np.asarray(x, np.float32)
                                .astype(ml_dtypes.bfloat16))


def kernel(**inp):
    x = np.asarray(inp["x"], np.float32)
    ei = np.asarray(inp["edge_index"], np.int64)
    N, IN = x.shape
    E = ei.shape[1]

    # self-loops are handled densely in launch B; streams carry real edges
    src = ei[0]
    dst = ei[1]

    pkey = ("plan", N, E, hash(ei.tobytes()))
    if pkey not in _CACHE:
        _CACHE[pkey] = build_plan(src, dst, N)
    shared, plans = _CACHE[pkey]

    def prep01(Wv, a_s, a_d, cb, sW, sb, g, b, m, v):
        Wv, sW = np.asarray(Wv, np.float32), np.asarray(sW, np.float32)
        bns = (np.asarray(g) / np.sqrt(np.asarray(v) + EPS)).astype(np.float32)
        bnt = (np.asarray(b) - np.asarray(m) * bns).astype(np.float32)
        Waug = np.concatenate(
            [Wv * bns[None, :], Wv @ _blockdiag(np.asarray(a_s)),
             Wv @ _blockdiag(np.asarray(a_d))], 1)
        return (Waug, sW * bns[None, :],
                (np.asarray(cb) + np.asarray(sb)) * bns + bnt, None)

    def prep2(Wv, a_s, a_d, cb, sW, sb):
        Wv = np.asarray(Wv, np.float32)
        Waug = np.concatenate(
            [np.eye(64, dtype=np.float32), Wv @ _blockdiag(np.asarray(a_s)),
             Wv @ _blockdiag(np.asarray(a_d))], 1)
        w2 = np.ascontiguousarray(
            Wv.reshape(64, 4, 64).transpose(1, 0, 2).reshape(256, 64)
            .reshape(2, 128, 64).transpose(1, 0, 2))
        return (Waug, np.asarray(sW, np.float32),
                np.asarray(cb) + np.asarray(sb), w2)

    Ls = [
        prep01(inp["conv0_W"], inp["conv0_as"], inp["conv0_ad"],
               inp["conv0_b"], inp["skip0_W"], inp["skip0_b"],
               inp["bn0_g"], inp["bn0_b"], inp["bn0_m"], inp["bn0_v"]),
        prep01(inp["conv1_W"], inp["conv1_as"], inp["conv1_ad"],
               inp["conv1_b"], inp["skip1_W"], inp["skip1_b"],
               inp["bn1_g"], inp["bn1_b"], inp["bn1_m"], inp["bn1_v"]),
        prep2(inp["conv2_W"], inp["conv2_as"], inp["conv2_ad"],
              inp["conv2_b"], inp["skip2_W"], inp["skip2_b"]),
    ]

    iota_np = np.tile(np.arange(P, dtype=np.float32), (P, 1)).astype(
        ml_dtypes.bfloat16)
    ident_np = np.eye(P, dtype=np.float32).astype(ml_dtypes.bfloat16)
    # sentinel row: zero feats, al_src = -40 (f32 packed in bf16 slots 64..71)
    sent_view = np.zeros(P, np.uint16)
    sent_view[64:72] = np.full(4, SENT_ALS, np.float32).view(np.uint16)
    sent = sent_view.view(ml_dtypes.bfloat16)

    h = x
    for li in range(3):
        F = IN if li == 0 else 64
        l2 = li == 2
        Waug, skipWf, biasv, w2 = Ls[li]
        akey = ("A", F)
        if akey not in _CACHE:
            _CACHE[akey] = build_dense(F)
        bkey = ("B", l2)
        if bkey not in _CACHE:
            _CACHE[bkey] = build_edge(shared, l2)

        hT = h.T.astype(np.float32)
        base_a = {
            "Waug": _bf(Waug),
            "skipW": _bf(skipWf),
            "biasR": np.tile(np.asarray(biasv, np.float32), (P, 1)),
        }
        a_maps = []
        for c in range(NC):
            node = c * SB + PERMC
            valid_s = PERMC < SB
            hts = np.zeros((F, DP), np.float32)
            hts[:, valid_s] = hT[:, node[valid_s]]
            vm = plans[c]["vmap"][PERMC]
            valid_d = vm >= 0
            htow = np.zeros((F, DP), np.float32)
            htow[:, valid_d] = hT[:, vm[valid_d]]
            a_maps.append(dict(base_a, hTs=_bf(hts), hTow=_bf(htow)))
        res_a = run_bass_kernel_spmd(_CACHE[akey], a_maps,
                                     core_ids=list(range(NC)))
        _RESULTS.append(res_a)
        if res_a.exec_time_ns:
            _EXEC_NS.append(res_a.exec_time_ns)

        tbl = np.empty((TROWS, P), ml_dtypes.bfloat16)
        for c in range(4):
            tbl[c * RB:(c + 1) * RB] = res_a.results[c]["tshard"]
        tbl[LO_ROWS] = sent
        for c in range(4, 8):
            tbl[LO_ROWS + 1 + (c - 4) * RB: LO_ROWS + 1 + (c - 3) * RB] = \
                res_a.results[c]["tshard"]
        tbl[TROWS - 1] = sent

        base_b = {"table": tbl, "iota": iota_np}
        if l2:
            base_b["w2"] = _bf(w2)
            base_b["ident"] = ident_np
        b_maps = []
        for c in range(NC):
            aldv = res_a.results[c]["aldv"]
            alde = aldv[plans[c]["slot_dst"]]  # [128, nchunks, 4]
            b_maps.append(dict(
                base_b,
                idx_lo=plans[c]["idx_lo"], idx_hi=plans[c]["idx_hi"],
                dstloc=plans[c]["dstloc"],
                alde=np.ascontiguousarray(alde.astype(np.float32)),
                skipd=np.ascontiguousarray(
                    res_a.results[c]["skipd"].astype(np.float32)),
                featd=np.ascontiguousarray(res_a.results[c]["featd"]),
                selfz=np.ascontiguousarray(
                    res_a.results[c]["selfz"].astype(np.float32))))
        res_b = run_bass_kernel_spmd(_CACHE[bkey], b_maps,
                                     core_ids=list(range(NC)))
        _RESULTS.append(res_b)
        if res_b.exec_time_ns:
            _EXEC_NS.append(res_b.exec_time_ns)

        hn = np.zeros((N, 64), np.float32)
        for c in range(NC):
            vm = plans[c]["vmap"]
            valid = vm >= 0
            hn[vm[valid]] = res_b.results[c]["y"][valid]
        h = hn
        _DBG.append(h)
    return h


# revision 38
# speedup vs baseline: 1.0196x; 1.0196x over previous
"""GAT (3-layer, PyG-style) Trainium2 Bass kernel, 8-core dst-sharded. v2.

Self-contained: takes full inputs, shards internally, returns full output.

Design (per layer -> two SPMD launches):
  Launch A (dense, src-sharded): each core computes its 1/8 of the gather
    table rows [64 feat bf16 | 4 al_src f32] = 256B/row via PE matmuls with
    Waug = [W(*bns) | W@blk(a_src) | W@blk(a_dst)], plus per-dst al_dst and
    the dense skip+bias rows for its dst shard. Host assembles the full
    table (plus 2 sentinel rows) and expands the per-slot al_dst stream.
  Launch B (edge, dst-sharded): exact-CSR edge slots (chunks of 128 slots,
    one (window, src-half) per chunk; schedule shared across cores, per-core
    counts balanced by an LPT assignment of dst nodes to (core, window)).
    Per group of chunks: dma_gather of 256B rows (4 SWDGE queues round-
    robin), ex = exp(leaky_relu(al_s + al_d)) on DVE/Act, messages scaled
    by ex, segment-sum via per-chunk one-hot matmuls (sel built on DVE by
    comparing a shipped dstloc column against an iota row) accumulating
    [msg | sum_ex] per 128-dst window in PSUM, drained into an SBUF
    accumulator. One batched normalize pass at the end.
  Layer 2 uses GAT linearity: segment-sum runs in h1-space (same 256B rows)
    with 4 per-head ex-scaled copies; W2 is applied per window afterwards
    (PE transpose + 2 matmuls), then mean over heads.
"""
import numpy as np
import ml_dtypes

import concourse.bacc as bacc
import concourse.mybir as mybir
import concourse.tile as tile
from concourse.alu_op_type import AluOpType
from concourse.bass_utils import run_bass_kernel_spmd

BF16 = mybir.dt.bfloat16
F32 = mybir.dt.float32
I16 = mybir.dt.int16

NC = 8
P = 128
EPS = 1e-5
SENT_ALS = -40.0
NQ = 4          # SWDGE queues
GC01 = 68       # chunks per gather group, layers 0/1
GC2 = 40        # chunks per gather group, layer 2

N_NODES = 50000
SB = N_NODES // NC          # src nodes per core (6250)
NWIN = 51                   # dst windows per core
WCAP = 125                  # dsts per window (< 128 for ceil slack)
DP = NWIN * P               # padded dst slots per core (6528)
RB = DP                     # table row block per core (6272)
LO_ROWS = 4 * RB            # rows in the lo half (25088)
SENT_REL = LO_ROWS          # sentinel index relative to each half base
TROWS = 2 * (LO_ROWS + 1)


def _ceil(a, b):
    return -(-a // b)


# ----------------------------------------------------------------- planning

def build_plan(src, dst, N):
    assert N == N_NODES
    # src -> permuted table row (relative to half base) and half flag
    s_core = src // SB
    arow = (src % SB) + s_core * RB
    half = (s_core >= 4).astype(np.int64)
    rel = np.where(half == 0, arow, arow - 4 * RB)

    # balanced assignment of dst nodes to (core, window) buckets, keeping
    # BOTH halves' per-bucket loads even (ceil(max_c/128) drives slot count)
    deg = np.zeros((N, 2), np.int64)
    np.add.at(deg, (dst, half), 1)
    tot = deg.sum(1)
    order = np.argsort(-tot, kind="stable")
    NB = NC * NWIN
    LO = np.zeros(NB, np.float64)
    HI = np.zeros(NB, np.float64)
    bcount = np.zeros(NB, np.int64)
    BIG = 1e18
    dst_c = np.empty(N, np.int64)
    dst_w = np.empty(N, np.int64)
    dst_p = np.empty(N, np.int64)
    for n in order:
        score = np.maximum(LO + deg[n, 0], HI + deg[n, 1])
        b = int(np.argmin(score))
        dst_c[n] = b // NWIN
        dst_w[n] = b % NWIN
        dst_p[n] = bcount[b]
        bcount[b] += 1
        LO[b] += deg[n, 0]
        HI[b] += deg[n, 1]
        if bcount[b] >= WCAP:
            LO[b] = HI[b] = BIG

    vmap = np.full((NC, DP), -1, np.int64)
    vmap[dst_c, dst_w * P + dst_p] = np.arange(N)

    # per (core, window, half) edge counts -> shared chunk schedule
    ec = dst_c[dst]
    ew = dst_w[dst]
    epos = dst_p[dst]
    cnt = np.zeros((NC, NWIN, 2), np.int64)
    np.add.at(cnt, (ec, ew, half), 1)
    nch = np.zeros((NWIN, 2), np.int64)
    for w in range(NWIN):
        for h in range(2):
            nch[w, h] = _ceil(int(cnt[:, w, h].max()), P) if cnt[:, w, h].max() else 0
        if nch[w].sum() == 0:
            nch[w, 0] = 1  # keep every window on the schedule
    chunks = []          # (w, h) in schedule order: lo sweep then hi sweep
    for h in range(2):
        for w in range(NWIN):
            chunks += [(w, h)] * int(nch[w, h])
    nchunks = len(chunks)
    slot_base = {}       # (w, h) -> first slot index in the combined stream
    o = 0
    for (w, h) in chunks:
        slot_base.setdefault((w, h), o * P)
        o += 1
    nlo = int(nch[:, 0].sum())
    SLO, SHI = nlo * P, (nchunks - nlo) * P

    # per-core streams
    plans = []
    for c in range(NC):
        em = ec == c
        eh = half[em]
        key = eh * (NWIN * P) + ew[em] * P  # per (h, w) group key base
        # stable ordering by (h, w); position within group via argsort
        si = np.argsort(key, kind="stable")
        erel = rel[em][si]
        ewk = ew[em][si]
        ehk = eh[si]
        epk = epos[em][si]
        idx_all = np.full(nchunks * P, SENT_REL, np.int64)
        dloc = np.full(nchunks * P, -1.0, np.float32)
        sdst = np.zeros(nchunks * P, np.int64)
        # group runs: edges sorted by (h, w); fill each group's slot range
        bounds = np.searchsorted(
            ehk * NWIN + ewk, np.arange(2 * NWIN + 1))
        for h in range(2):
            for w in range(NWIN):
                a, b = bounds[h * NWIN + w], bounds[h * NWIN + w + 1]
                if a == b:
                    continue
                s0 = slot_base[(w, h)]
                m = b - a
                idx_all[s0: s0 + m] = erel[a:b]
                dloc[s0: s0 + m] = epk[a:b]
                sdst[s0: s0 + m] = w * P + epk[a:b]
        idx_lo = _wrap16(idx_all[:SLO])
        idx_hi = _wrap16(idx_all[SLO:])
        dstloc = dloc.reshape(nchunks, P).T.astype(ml_dtypes.bfloat16)
        sdst_w = sdst.reshape(nchunks, P).transpose(1, 0)
        plans.append(dict(vmap=vmap[c], idx_lo=idx_lo, idx_hi=idx_hi,
                          dstloc=np.ascontiguousarray(dstloc),
                          slot_dst=np.ascontiguousarray(sdst_w)))
    shared = dict(chunks=chunks, nchunks=nchunks, nlo=nlo, SLO=SLO, SHI=SHI)
    return shared, plans


def _wrap16(stream):
    S = len(stream)
    w = stream.reshape(S // 16, 16).T.astype(np.int16)
    return np.ascontiguousarray(np.tile(w, (8, 1)))


PERMC = (np.arange(DP) % P) * NWIN + np.arange(DP) // P  # col k -> slot p*49+i


# ------------------------------------------------------------- launch A (dense)

def build_dense(F):
    nc = bacc.Bacc("TRN2", target_bir_lowering=False, debug=False)
    hTs = nc.dram_tensor("hTs", [F, DP], BF16, kind="ExternalInput")
    hTow = nc.dram_tensor("hTow", [F, DP], BF16, kind="ExternalInput")
    Waug = nc.dram_tensor("Waug", [F, 72], BF16, kind="ExternalInput")
    skipW = nc.dram_tensor("skipW", [F, 64], BF16, kind="ExternalInput")
    biasR = nc.dram_tensor("biasR", [P, 64], F32, kind="ExternalInput")
    tshard = nc.dram_tensor("tshard", [DP, P], BF16, kind="ExternalOutput")
    aldv = nc.dram_tensor("aldv", [DP, 4], F32, kind="ExternalOutput")
    skipd = nc.dram_tensor("skipd", [DP, 64], F32, kind="ExternalOutput")
    featd = nc.dram_tensor("featd", [DP, 64], BF16, kind="ExternalOutput")
    selfz = nc.dram_tensor("selfz", [DP, 4], F32, kind="ExternalOutput")

    with tile.TileContext(nc) as tc:
        with (
            tc.tile_pool(name="c", bufs=1) as cp,
            tc.tile_pool(name="ps", bufs=2, space="PSUM") as pp,
        ):
            hts_sb = cp.tile([F, DP], BF16)
            nc.sync.dma_start(hts_sb[:], hTs[:])
            htow_sb = cp.tile([F, DP], BF16)
            nc.sync.dma_start(htow_sb[:], hTow[:])
            waug_sb = cp.tile([F, 72], BF16)
            nc.sync.dma_start(waug_sb[:], Waug[:])
            skipw_sb = cp.tile([F, 64], BF16)
            nc.sync.dma_start(skipw_sb[:], skipW[:])
            bias_sb = cp.tile([P, 64], F32)
            nc.sync.dma_start(bias_sb[:], biasR[:])

            tstage = cp.tile([P, NWIN, P], BF16)
            asb = cp.tile([P, NWIN, 4], F32)
            ssb = cp.tile([P, NWIN, 64], F32)
            fsb = cp.tile([P, NWIN, 64], BF16)
            szb = cp.tile([P, NWIN, 4], F32)
            nc.gpsimd.memset(tstage[:], 0)
            for i in range(NWIN):
                dps = pp.tile([P, 72], F32, space="PSUM", tag="dps")
                nc.tensor.matmul(dps[:], hts_sb[:, i * P:(i + 1) * P],
                                 waug_sb[:], start=True, stop=True)
                nc.vector.tensor_copy(tstage[:, i, 0:64], dps[:, 0:64])
                tf32 = tstage[:].bitcast(F32)
                nc.vector.tensor_copy(tf32[:, i, 32:36], dps[:, 64:68])
                ops_ = pp.tile([P, 72], F32, space="PSUM", tag="ops")
                nc.tensor.matmul(ops_[:], htow_sb[:, i * P:(i + 1) * P],
                                 waug_sb[:], start=True, stop=True)
                sps = pp.tile([P, 64], F32, space="PSUM", tag="sps")
                nc.tensor.matmul(sps[:], htow_sb[:, i * P:(i + 1) * P],
                                 skipw_sb[:], start=True, stop=True)
                nc.vector.tensor_copy(asb[:, i, :], ops_[:, 68:72])
                nc.vector.tensor_copy(fsb[:, i, :], ops_[:, 0:64])
                nc.vector.tensor_tensor(szb[:, i, :], ops_[:, 64:68],
                                        asb[:, i, :], AluOpType.add)
                nc.vector.tensor_tensor(ssb[:, i, :], sps[:],
                                        bias_sb[:], AluOpType.add)
            nc.sync.dma_start(
                tshard[:].rearrange("(p i) w -> p i w", p=P), tstage[:])
            nc.sync.dma_start(
                aldv[:].rearrange("(p i) h -> p i h", p=P), asb[:])
            nc.sync.dma_start(
                skipd[:].rearrange("(p i) c -> p i c", p=P), ssb[:])
            nc.sync.dma_start(
                featd[:].rearrange("(p i) c -> p i c", p=P), fsb[:])
            nc.sync.dma_start(
                selfz[:].rearrange("(p i) h -> p i h", p=P), szb[:])
    nc.compile()
    return nc


# ------------------------------------------------------------- launch B (edge)

def build_edge(shared, l2):
    chunks = shared["chunks"]
    nchunks = shared["nchunks"]
    nlo = shared["nlo"]
    SLO, SHI = shared["SLO"], shared["SHI"]
    GC = GC2 if l2 else GC01
    NW = 260 if l2 else 68

    nc = bacc.Bacc("TRN2", target_bir_lowering=False, debug=False,
                   num_swdge_queues=NQ)
    table = nc.dram_tensor("table", [TROWS, P], BF16, kind="ExternalInput")
    idx_lo = nc.dram_tensor("idx_lo", [P, max(SLO, 16) // 16], I16,
                            kind="ExternalInput")
    idx_hi = nc.dram_tensor("idx_hi", [P, max(SHI, 16) // 16], I16,
                            kind="ExternalInput")
    dstloc = nc.dram_tensor("dstloc", [P, nchunks], BF16,
                            kind="ExternalInput")
    alde_in = nc.dram_tensor("alde", [P, nchunks, 4], F32,
                             kind="ExternalInput")
    skipd_in = nc.dram_tensor("skipd", [DP, 64], F32, kind="ExternalInput")
    featd_in = nc.dram_tensor("featd", [DP, 64], BF16, kind="ExternalInput")
    selfz_in = nc.dram_tensor("selfz", [DP, 4], F32, kind="ExternalInput")
    iota_in = nc.dram_tensor("iota", [P, P], BF16, kind="ExternalInput")
    if l2:
        w2_in = nc.dram_tensor("w2", [P, 2, 64], BF16, kind="ExternalInput")
        ident_in = nc.dram_tensor("ident", [P, P], BF16, kind="ExternalInput")
    y_out = nc.dram_tensor("y", [DP, 64], F32, kind="ExternalOutput")

    # group schedule: runs of <= GC chunks, same half
    groups = []
    k = 0
    while k < nchunks:
        k1 = min(k + GC, nchunks, nlo if k < nlo else nchunks)
        groups.append((k, k1))
        k = k1
    first = [False] * nchunks
    last = [False] * nchunks
    wlast = [False] * nchunks
    seen = set()
    wl = {}
    for k, (w, h) in enumerate(chunks):
        if (h, w) not in seen:
            seen.add((h, w))
            first[k] = True
        if k + 1 >= nchunks or chunks[k + 1] != (w, h):
            last[k] = True
        wl[w] = k
    for w, k in wl.items():
        wlast[k] = True

    with tile.TileContext(nc) as tc:
        with (
            tc.tile_pool(name="c", bufs=1) as cp,
            tc.tile_pool(name="g", bufs=3 if l2 else 4) as gp,
            tc.tile_pool(name="r", bufs=2) as rp,
            tc.tile_pool(name="s", bufs=3) as sp,
            tc.tile_pool(name="pw", bufs=4, space="PSUM") as pw,
            tc.tile_pool(name="pt", bufs=2, space="PSUM") as pt,
        ):
            # big constant loads go on the Act engine's DGE so the first
            # gather groups' idx loads (sync engine) aren't queued behind them
            dloc_sb = cp.tile([P, nchunks], BF16)
            nc.scalar.dma_start(dloc_sb[:], dstloc[:])
            alde_full = cp.tile([P, nchunks, 4], F32)
            nc.scalar.dma_start(alde_full[:], alde_in[:])
            skipd_sb = cp.tile([P, NWIN, 64], F32)
            nc.scalar.dma_start(
                skipd_sb[:], skipd_in[:].rearrange("(i p) c -> p i c", p=P))
            featd_sb = cp.tile([P, NWIN, 64], BF16)
            nc.scalar.dma_start(
                featd_sb[:], featd_in[:].rearrange("(i p) c -> p i c", p=P))
            selfz_sb = cp.tile([P, NWIN, 4], F32)
            nc.scalar.dma_start(
                selfz_sb[:], selfz_in[:].rearrange("(i p) h -> p i h", p=P))
            iota_sb = cp.tile([P, 1, P], BF16)
            nc.scalar.dma_start(iota_sb[:, 0, :], iota_in[:])
            if l2:
                w2_sb = cp.tile([P, 2, 64], BF16)
                nc.scalar.dma_start(w2_sb[:], w2_in[:])
                ident_sb = cp.tile([P, P], BF16)
                nc.scalar.dma_start(ident_sb[:], ident_in[:])
            msum = cp.tile([P, NWIN, NW], F32)
            y_sb = cp.tile([P, NWIN, 64], F32)

            # init msum with the dense self-loop contributions
            exself = cp.tile([P, NWIN, 4], F32)
            nc.vector.scalar_tensor_tensor(
                exself[:], selfz_sb[:], 0.2, selfz_sb[:],
                AluOpType.mult, AluOpType.max)
            nc.scalar.activation(exself[:], exself[:],
                                 mybir.ActivationFunctionType.Exp)
            nc.vector.tensor_copy(msum[:, :, NW - 4: NW], exself[:])
            if l2:
                nc.vector.tensor_tensor(
                    msum[:, :, 0:256].rearrange("p w (h c) -> p w h c", c=64),
                    featd_sb[:, :, None, :].to_broadcast([P, NWIN, 4, 64]),
                    exself[:, :, :, None].to_broadcast([P, NWIN, 4, 64]),
                    AluOpType.mult)
            else:
                nc.vector.tensor_tensor(
                    msum[:, :, 0:64].rearrange("p w (h c) -> p w h c", c=16),
                    featd_sb[:].rearrange("p w (h c) -> p w h c", c=16),
                    exself[:, :, :, None].to_broadcast([P, NWIN, 4, 16]),
                    AluOpType.mult)

            win_ps = {}
            for gi, (k0, k1) in enumerate(groups):
                T = k1 - k0
                h = chunks[k0][1]
                base = table[0: LO_ROWS + 1, :] if h == 0 \
                    else table[LO_ROWS + 1: TROWS, :]
                o16 = (k0 * P if h == 0 else (k0 - nlo) * P) // 16
                idx_t = sp.tile([P, GC * 8], I16, tag="idx")
                nc.sync.dma_start(
                    idx_t[:, : T * 8],
                    (idx_lo if h == 0 else idx_hi)[:, o16: o16 + T * 8])
                gt = gp.tile([P, GC, P], BF16, tag="g")
                nc.gpsimd.dma_gather(
                    gt[:, :T, :], base, idx_t[:, : T * 8], T * P, T * P, P,
                    single_packet=False, queue_num=gi % NQ)

                zf = sp.tile([P, GC, 4], F32, tag="z")
                gf = gt[:].bitcast(F32)
                nc.vector.tensor_tensor(zf[:, :T, :], gf[:, :T, 32:36],
                                        alde_full[:, k0:k1, :], AluOpType.add)
                nc.vector.scalar_tensor_tensor(
                    zf[:, :T, :], zf[:, :T, :], 0.2, zf[:, :T, :],
                    AluOpType.mult, AluOpType.max)
                sel = sp.tile([P, GC, P], BF16, tag="sel")
                nc.vector.tensor_tensor(
                    sel[:, :T, :],
                    iota_sb[:].to_broadcast([P, T, P]),
                    dloc_sb[:, k0:k1, None].to_broadcast([P, T, P]),
                    AluOpType.is_equal)

                if l2:
                    rhs = rp.tile([P, GC, 260], BF16, tag="rhs")
                    nc.scalar.activation(rhs[:, :T, 256:260], zf[:, :T, :],
                                         mybir.ActivationFunctionType.Exp)
                    nc.vector.tensor_tensor(
                        rhs[:, :T, 0:256].rearrange(
                            "p t (h c) -> p t h c", c=64),
                        gt[:, :T, None, 0:64].to_broadcast([P, T, 4, 64]),
                        rhs[:, :T, 256:260, None].to_broadcast([P, T, 4, 64]),
                        AluOpType.mult)
                else:
                    nc.scalar.activation(gt[:, :T, 64:68], zf[:, :T, :],
                                         mybir.ActivationFunctionType.Exp)
                    nc.vector.tensor_tensor(
                        gt[:, :T, 0:64].rearrange("p t (h c) -> p t h c", c=16),
                        gt[:, :T, 0:64].rearrange("p t (h c) -> p t h c", c=16),
                        gt[:, :T, 64:68, None].to_broadcast([P, T, 4, 16]),
                        AluOpType.mult)

                for t in range(T):
                    k = k0 + t
                    w, hh = chunks[k]
                    if first[k]:
                        win_ps[w] = pw.tile([P, NW], F32, space="PSUM",
                                            tag="win", name=f"win{w}h{hh}")
                    rhs_ap = rhs[:, t, :] if l2 else gt[:, t, 0:68]
                    nc.tensor.matmul(win_ps[w][:], sel[:, t, :], rhs_ap,
                                     start=first[k], stop=last[k],
                                     skip_group_check=True)
                    if last[k]:
                        pwin = win_ps.pop(w)
                        nc.vector.tensor_tensor(msum[:, w, :], msum[:, w, :],
                                                pwin[:], AluOpType.add)
                    if l2 and wlast[k]:
                        # per-window W2 drain, overlapped with later groups
                        recw = sp.tile([P, 4], F32, tag="recw")
                        nc.vector.reciprocal(recw[:], msum[:, w, 256:260])
                        snw = sp.tile([P, 4, 64], BF16, tag="snw")
                        nc.vector.tensor_tensor(
                            snw[:],
                            msum[:, w, 0:256].rearrange(
                                "p (h c) -> p h c", c=64),
                            recw[:, :, None].to_broadcast([P, 4, 64]),
                            AluOpType.mult)
                        yps = pt.tile([P, 64], F32, space="PSUM", tag="yps")
                        for j in range(2):
                            tp = pt.tile([P, P], BF16, space="PSUM", tag="tp")
                            nc.tensor.matmul(
                                tp[:],
                                snw[:].rearrange("p h c -> p (h c)")
                                      [:, j * P:(j + 1) * P],
                                ident_sb[:], is_transpose=True,
                                start=True, stop=True, skip_group_check=True)
                            st = sp.tile([P, P], BF16, tag="st")
                            nc.scalar.activation(
                                st[:], tp[:],
                                mybir.ActivationFunctionType.Copy)
                            nc.tensor.matmul(yps[:], st[:], w2_sb[:, j, :],
                                             start=(j == 0), stop=(j == 1),
                                             skip_group_check=True)
                        nc.vector.scalar_tensor_tensor(
                            y_sb[:, w, :], yps[:], 0.25, skipd_sb[:, w, :],
                            AluOpType.mult, AluOpType.add)

            if not l2:
                rec = cp.tile([P, NWIN, 4], F32)
                nc.vector.reciprocal(rec[:], msum[:, :, 64:68])
                nc.vector.tensor_tensor(
                    y_sb[:].rearrange("p w (h c) -> p w h c", c=16),
                    msum[:, :, 0:64].rearrange("p w (h c) -> p w h c", c=16),
                    rec[:, :, :, None].to_broadcast([P, NWIN, 4, 16]),
                    AluOpType.mult)
                nc.vector.tensor_tensor(y_sb[:], y_sb[:], skipd_sb[:],
                                        AluOpType.add)
                nc.vector.tensor_scalar_max(y_sb[:], y_sb[:], 0.0)
            nc.sync.dma_start(
                y_out[:].rearrange("(i p) c -> p i c", p=P), y_sb[:])
    nc.compile()
    return nc


# ------------------------------------------------------------------ driver

_CACHE = {}
_DBG = []
_EXEC_NS = []
_RESULTS = []


def _blockdiag(a):
    H, C = a.shape
    m = np.zeros((H * C, H), np.float32)
    for hh in range(H):
        m[hh * C: (hh + 1) * C, hh] = a[hh]
    return m


def _bf(x):
    return np.ascontiguousarray(np.asarray(x, np.float32)
                                .astype(ml_dtypes.bfloat16))


def kernel(**inp):
    x = np.asarray(inp["x"], np.float32)
    ei = np.asarray(inp["edge_index"], np.int64)
    N, IN = x.shape
    E = ei.shape[1]

    # self-loops are handled densely in launch B; streams carry real edges
    src = ei[0]
    dst = ei[1]

    pkey = ("plan", N, E, hash(ei.tobytes()))
    if pkey not in _CACHE:
        _CACHE[pkey] = build_plan(src, dst, N)
    shared, plans = _CACHE[pkey]

    def prep01(Wv, a_s, a_d, cb, sW, sb, g, b, m, v):
        Wv, sW = np.asarray(Wv, np.float32), np.asarray(sW, np.float32)
        bns = (np.asarray(g) / np.sqrt(np.asarray(v) + EPS)).astype(np.float32)
        bnt = (np.asarray(b) - np.asarray(m) * bns).astype(np.float32)
        Waug = np.concatenate(
            [Wv * bns[None, :], Wv @ _blockdiag(np.asarray(a_s)),
             Wv @ _blockdiag(np.asarray(a_d))], 1)
        return (Waug, sW * bns[None, :],
                (np.asarray(cb) + np.asarray(sb)) * bns + bnt, None)

    def prep2(Wv, a_s, a_d, cb, sW, sb):
        Wv = np.asarray(Wv, np.float32)
        Waug = np.concatenate(
            [np.eye(64, dtype=np.float32), Wv @ _blockdiag(np.asarray(a_s)),
             Wv @ _blockdiag(np.asarray(a_d))], 1)
        w2 = np.ascontiguousarray(
            Wv.reshape(64, 4, 64).transpose(1, 0, 2).reshape(256, 64)
            .reshape(2, 128, 64).transpose(1, 0, 2))
        return (Waug, np.asarray(sW, np.float32),
                np.asarray(cb) + np.asarray(sb), w2)

    Ls = [
        prep01(inp["conv0_W"], inp["conv0_as"], inp["conv0_ad"],
               inp["conv0_b"], inp["skip0_W"], inp["skip0_b"],
               inp["bn0_g"], inp["bn0_b"], inp["bn0_m"], inp["bn0_v"]),
        prep01(inp["conv1_W"], inp["conv1_as"], inp["conv1_ad"],
               inp["conv1_b"], inp["skip1_W"], inp["skip1_b"],
               inp["bn1_g"], inp["bn1_b"], inp["bn1_m"], inp["bn1_v"]),
        prep2(inp["conv2_W"], inp["conv2_as"], inp["conv2_ad"],
              inp["conv2_b"], inp["skip2_W"], inp["skip2_b"]),
    ]

    iota_np = np.tile(np.arange(P, dtype=np.float32), (P, 1)).astype(
        ml_dtypes.bfloat16)
    ident_np = np.eye(P, dtype=np.float32).astype(ml_dtypes.bfloat16)
    # sentinel row: zero feats, al_src = -40 (f32 packed in bf16 slots 64..71)
    sent_view = np.zeros(P, np.uint16)
    sent_view[64:72] = np.full(4, SENT_ALS, np.float32).view(np.uint16)
    sent = sent_view.view(ml_dtypes.bfloat16)

    h = x
    for li in range(3):
        F = IN if li == 0 else 64
        l2 = li == 2
        Waug, skipWf, biasv, w2 = Ls[li]
        akey = ("A", F)
        if akey not in _CACHE:
            _CACHE[akey] = build_dense(F)
        bkey = ("B", l2, pkey)  # schedule is baked into the B kernel
        if bkey not in _CACHE:
            _CACHE[bkey] = build_edge(shared, l2)

        hT = h.T.astype(np.float32)
        base_a = {
            "Waug": _bf(Waug),
            "skipW": _bf(skipWf),
            "biasR": np.tile(np.asarray(biasv, np.float32), (P, 1)),
        }
        a_maps = []
        for c in range(NC):
            node = c * SB + PERMC
            valid_s = PERMC < SB
            hts = np.zeros((F, DP), np.float32)
            hts[:, valid_s] = hT[:, node[valid_s]]
            vm = plans[c]["vmap"][PERMC]
            valid_d = vm >= 0
            htow = np.zeros((F, DP), np.float32)
            htow[:, valid_d] = hT[:, vm[valid_d]]
            a_maps.append(dict(base_a, hTs=_bf(hts), hTow=_bf(htow)))
        res_a = run_bass_kernel_spmd(_CACHE[akey], a_maps,
                                     core_ids=list(range(NC)))
        _RESULTS.append(res_a)
        if res_a.exec_time_ns:
            _EXEC_NS.append(res_a.exec_time_ns)

        tbl = np.empty((TROWS, P), ml_dtypes.bfloat16)
        for c in range(4):
            tbl[c * RB:(c + 1) * RB] = res_a.results[c]["tshard"]
        tbl[LO_ROWS] = sent
        for c in range(4, 8):
            tbl[LO_ROWS + 1 + (c - 4) * RB: LO_ROWS + 1 + (c - 3) * RB] = \
                res_a.results[c]["tshard"]
        tbl[TROWS - 1] = sent

        base_b = {"table": tbl, "iota": iota_np}
        if l2:
            base_b["w2"] = _bf(w2)
            base_b["ident"] = ident_np
        b_maps = []
        for c in range(NC):
            aldv = res_a.results[c]["aldv"]
            alde = aldv[plans[c]["slot_dst"]]  # [128, nchunks, 4]
            b_maps.append(dict(
                base_b,
                idx_lo=plans[c]["idx_lo"], idx_hi=plans[c]["idx_hi"],
                dstloc=plans[c]["dstloc"],
                alde=np.ascontiguousarray(alde.astype(np.float32)),
                skipd=np.ascontiguousarray(
                    res_a.results[c]["skipd"].astype(np.float32)),
                featd=np.ascontiguousarray(res_a.results[c]["featd"]),
                selfz=np.ascontiguousarray(
                    res_a.results[c]["selfz"].astype(np.float32))))
        res_b = run_bass_kernel_spmd(_CACHE[bkey], b_maps,
                                     core_ids=list(range(NC)))
        _RESULTS.append(res_b)
        if res_b.exec_time_ns:
            _EXEC_NS.append(res_b.exec_time_ns)

        hn = np.zeros((N, 64), np.float32)
        for c in range(NC):
            vm = plans[c]["vmap"]
            valid = vm >= 0
            hn[vm[valid]] = res_b.results[c]["y"][valid]
        h = hn
        _DBG.append(h)
    return h


# revision 41
# speedup vs baseline: 1.0219x; 1.0023x over previous
"""GAT (3-layer, PyG-style) Trainium2 Bass kernel, 8-core dst-sharded. v2.

Self-contained: takes full inputs, shards internally, returns full output.

Design (per layer -> two SPMD launches):
  Launch A (dense, src-sharded): each core computes its 1/8 of the gather
    table rows [64 feat bf16 | 4 al_src f32] = 256B/row via PE matmuls with
    Waug = [W(*bns) | W@blk(a_src) | W@blk(a_dst)], plus per-dst al_dst and
    the dense skip+bias rows for its dst shard. Host assembles the full
    table (plus 2 sentinel rows) and expands the per-slot al_dst stream.
  Launch B (edge, dst-sharded): exact-CSR edge slots (chunks of 128 slots,
    one (window, src-half) per chunk; schedule shared across cores, per-core
    counts balanced by an LPT assignment of dst nodes to (core, window)).
    Per group of chunks: dma_gather of 256B rows (4 SWDGE queues round-
    robin), ex = exp(leaky_relu(al_s + al_d)) on DVE/Act, messages scaled
    by ex, segment-sum via per-chunk one-hot matmuls (sel built on DVE by
    comparing a shipped dstloc column against an iota row) accumulating
    [msg | sum_ex] per 128-dst window in PSUM, drained into an SBUF
    accumulator. One batched normalize pass at the end.
  Layer 2 uses GAT linearity: segment-sum runs in h1-space (same 256B rows)
    with 4 per-head ex-scaled copies; W2 is applied per window afterwards
    (PE transpose + 2 matmuls), then mean over heads.
"""
import numpy as np
import ml_dtypes

import concourse.bacc as bacc
import concourse.mybir as mybir
import concourse.tile as tile
from concourse.alu_op_type import AluOpType
from concourse.bass_utils import run_bass_kernel_spmd

BF16 = mybir.dt.bfloat16
F32 = mybir.dt.float32
I16 = mybir.dt.int16

NC = 8
P = 128
EPS = 1e-5
SENT_ALS = -40.0
NQ = 4          # SWDGE queues
GC01 = 68       # chunks per gather group, layers 0/1
GC2 = 51        # chunks per gather group, layer 2

N_NODES = 50000
SB = N_NODES // NC          # src nodes per core (6250)
NWIN = 51                   # dst windows per core
WCAP = 125                  # dsts per window (< 128 for ceil slack)
DP = NWIN * P               # padded dst slots per core (6528)
RB = DP                     # table row block per core (6272)
LO_ROWS = 4 * RB            # rows in the lo half (25088)
SENT_REL = LO_ROWS          # sentinel index relative to each half base
TROWS = 2 * (LO_ROWS + 1)


def _ceil(a, b):
    return -(-a // b)


# ----------------------------------------------------------------- planning

def build_plan(src, dst, N):
    assert N == N_NODES
    # src -> permuted table row (relative to half base) and half flag
    s_core = src // SB
    arow = (src % SB) + s_core * RB
    half = (s_core >= 4).astype(np.int64)
    rel = np.where(half == 0, arow, arow - 4 * RB)

    # balanced assignment of dst nodes to (core, window) buckets, keeping
    # BOTH halves' per-bucket loads even (ceil(max_c/128) drives slot count)
    deg = np.zeros((N, 2), np.int64)
    np.add.at(deg, (dst, half), 1)
    tot = deg.sum(1)
    order = np.argsort(-tot, kind="stable")
    NB = NC * NWIN
    LO = np.zeros(NB, np.float64)
    HI = np.zeros(NB, np.float64)
    bcount = np.zeros(NB, np.int64)
    BIG = 1e18
    dst_c = np.empty(N, np.int64)
    dst_w = np.empty(N, np.int64)
    dst_p = np.empty(N, np.int64)
    for n in order:
        score = np.maximum(LO + deg[n, 0], HI + deg[n, 1])
        b = int(np.argmin(score))
        dst_c[n] = b // NWIN
        dst_w[n] = b % NWIN
        dst_p[n] = bcount[b]
        bcount[b] += 1
        LO[b] += deg[n, 0]
        HI[b] += deg[n, 1]
        if bcount[b] >= WCAP:
            LO[b] = HI[b] = BIG

    vmap = np.full((NC, DP), -1, np.int64)
    vmap[dst_c, dst_w * P + dst_p] = np.arange(N)

    # per (core, window, half) edge counts -> shared chunk schedule
    ec = dst_c[dst]
    ew = dst_w[dst]
    epos = dst_p[dst]
    cnt = np.zeros((NC, NWIN, 2), np.int64)
    np.add.at(cnt, (ec, ew, half), 1)
    nch = np.zeros((NWIN, 2), np.int64)
    for w in range(NWIN):
        for h in range(2):
            nch[w, h] = _ceil(int(cnt[:, w, h].max()), P) if cnt[:, w, h].max() else 0
        if nch[w].sum() == 0:
            nch[w, 0] = 1  # keep every window on the schedule
    chunks = []          # (w, h) in schedule order: lo sweep then hi sweep
    for h in range(2):
        for w in range(NWIN):
            chunks += [(w, h)] * int(nch[w, h])
    nchunks = len(chunks)
    slot_base = {}       # (w, h) -> first slot index in the combined stream
    o = 0
    for (w, h) in chunks:
        slot_base.setdefault((w, h), o * P)
        o += 1
    nlo = int(nch[:, 0].sum())
    SLO, SHI = nlo * P, (nchunks - nlo) * P

    # per-core streams
    plans = []
    for c in range(NC):
        em = ec == c
        eh = half[em]
        key = eh * (NWIN * P) + ew[em] * P  # per (h, w) group key base
        # stable ordering by (h, w); position within group via argsort
        si = np.argsort(key, kind="stable")
        erel = rel[em][si]
        ewk = ew[em][si]
        ehk = eh[si]
        epk = epos[em][si]
        idx_all = np.full(nchunks * P, SENT_REL, np.int64)
        dloc = np.full(nchunks * P, -1.0, np.float32)
        sdst = np.zeros(nchunks * P, np.int64)
        # group runs: edges sorted by (h, w); fill each group's slot range
        bounds = np.searchsorted(
            ehk * NWIN + ewk, np.arange(2 * NWIN + 1))
        for h in range(2):
            for w in range(NWIN):
                a, b = bounds[h * NWIN + w], bounds[h * NWIN + w + 1]
                if a == b:
                    continue
                s0 = slot_base[(w, h)]
                m = b - a
                idx_all[s0: s0 + m] = erel[a:b]
                dloc[s0: s0 + m] = epk[a:b]
                sdst[s0: s0 + m] = w * P + epk[a:b]
        idx_lo = _wrap16(idx_all[:SLO])
        idx_hi = _wrap16(idx_all[SLO:])
        dstloc = dloc.reshape(nchunks, P).T.astype(ml_dtypes.bfloat16)
        sdst_w = sdst.reshape(nchunks, P).transpose(1, 0)
        plans.append(dict(vmap=vmap[c], idx_lo=idx_lo, idx_hi=idx_hi,
                          dstloc=np.ascontiguousarray(dstloc),
                          slot_dst=np.ascontiguousarray(sdst_w)))
    shared = dict(chunks=chunks, nchunks=nchunks, nlo=nlo, SLO=SLO, SHI=SHI)
    return shared, plans


def _wrap16(stream):
    S = len(stream)
    w = stream.reshape(S // 16, 16).T.astype(np.int16)
    return np.ascontiguousarray(np.tile(w, (8, 1)))


PERMC = (np.arange(DP) % P) * NWIN + np.arange(DP) // P  # col k -> slot p*49+i


# ------------------------------------------------------------- launch A (dense)

def build_dense(F):
    nc = bacc.Bacc("TRN2", target_bir_lowering=False, debug=False)
    hTs = nc.dram_tensor("hTs", [F, DP], BF16, kind="ExternalInput")
    hTow = nc.dram_tensor("hTow", [F, DP], BF16, kind="ExternalInput")
    Waug = nc.dram_tensor("Waug", [F, 72], BF16, kind="ExternalInput")
    skipW = nc.dram_tensor("skipW", [F, 64], BF16, kind="ExternalInput")
    biasR = nc.dram_tensor("biasR", [P, 64], F32, kind="ExternalInput")
    tshard = nc.dram_tensor("tshard", [DP, P], BF16, kind="ExternalOutput")
    aldv = nc.dram_tensor("aldv", [DP, 4], F32, kind="ExternalOutput")
    skipd = nc.dram_tensor("skipd", [DP, 64], F32, kind="ExternalOutput")
    featd = nc.dram_tensor("featd", [DP, 64], BF16, kind="ExternalOutput")
    selfz = nc.dram_tensor("selfz", [DP, 4], F32, kind="ExternalOutput")

    with tile.TileContext(nc) as tc:
        with (
            tc.tile_pool(name="c", bufs=1) as cp,
            tc.tile_pool(name="ps", bufs=2, space="PSUM") as pp,
        ):
            hts_sb = cp.tile([F, DP], BF16)
            nc.sync.dma_start(hts_sb[:], hTs[:])
            htow_sb = cp.tile([F, DP], BF16)
            nc.sync.dma_start(htow_sb[:], hTow[:])
            waug_sb = cp.tile([F, 72], BF16)
            nc.sync.dma_start(waug_sb[:], Waug[:])
            skipw_sb = cp.tile([F, 64], BF16)
            nc.sync.dma_start(skipw_sb[:], skipW[:])
            bias_sb = cp.tile([P, 64], F32)
            nc.sync.dma_start(bias_sb[:], biasR[:])

            tstage = cp.tile([P, NWIN, P], BF16)
            asb = cp.tile([P, NWIN, 4], F32)
            ssb = cp.tile([P, NWIN, 64], F32)
            fsb = cp.tile([P, NWIN, 64], BF16)
            szb = cp.tile([P, NWIN, 4], F32)
            nc.gpsimd.memset(tstage[:], 0)
            for i in range(NWIN):
                dps = pp.tile([P, 72], F32, space="PSUM", tag="dps")
                nc.tensor.matmul(dps[:], hts_sb[:, i * P:(i + 1) * P],
                                 waug_sb[:], start=True, stop=True)
                nc.vector.tensor_copy(tstage[:, i, 0:64], dps[:, 0:64])
                tf32 = tstage[:].bitcast(F32)
                nc.vector.tensor_copy(tf32[:, i, 32:36], dps[:, 64:68])
                ops_ = pp.tile([P, 72], F32, space="PSUM", tag="ops")
                nc.tensor.matmul(ops_[:], htow_sb[:, i * P:(i + 1) * P],
                                 waug_sb[:], start=True, stop=True)
                sps = pp.tile([P, 64], F32, space="PSUM", tag="sps")
                nc.tensor.matmul(sps[:], htow_sb[:, i * P:(i + 1) * P],
                                 skipw_sb[:], start=True, stop=True)
                nc.vector.tensor_copy(asb[:, i, :], ops_[:, 68:72])
                nc.vector.tensor_copy(fsb[:, i, :], ops_[:, 0:64])
                nc.vector.tensor_tensor(szb[:, i, :], ops_[:, 64:68],
                                        asb[:, i, :], AluOpType.add)
                nc.vector.tensor_tensor(ssb[:, i, :], sps[:],
                                        bias_sb[:], AluOpType.add)
            nc.sync.dma_start(
                tshard[:].rearrange("(p i) w -> p i w", p=P), tstage[:])
            nc.sync.dma_start(
                aldv[:].rearrange("(p i) h -> p i h", p=P), asb[:])
            nc.sync.dma_start(
                skipd[:].rearrange("(p i) c -> p i c", p=P), ssb[:])
            nc.sync.dma_start(
                featd[:].rearrange("(p i) c -> p i c", p=P), fsb[:])
            nc.sync.dma_start(
                selfz[:].rearrange("(p i) h -> p i h", p=P), szb[:])
    nc.compile()
    return nc


# ------------------------------------------------------------- launch B (edge)

def build_edge(shared, l2):
    chunks = shared["chunks"]
    nchunks = shared["nchunks"]
    nlo = shared["nlo"]
    SLO, SHI = shared["SLO"], shared["SHI"]
    GC = GC2 if l2 else GC01
    NW = 260 if l2 else 68

    nc = bacc.Bacc("TRN2", target_bir_lowering=False, debug=False,
                   num_swdge_queues=NQ)
    table = nc.dram_tensor("table", [TROWS, P], BF16, kind="ExternalInput")
    idx_lo = nc.dram_tensor("idx_lo", [P, max(SLO, 16) // 16], I16,
                            kind="ExternalInput")
    idx_hi = nc.dram_tensor("idx_hi", [P, max(SHI, 16) // 16], I16,
                            kind="ExternalInput")
    dstloc = nc.dram_tensor("dstloc", [P, nchunks], BF16,
                            kind="ExternalInput")
    alde_in = nc.dram_tensor("alde", [P, nchunks, 4], F32,
                             kind="ExternalInput")
    skipd_in = nc.dram_tensor("skipd", [DP, 64], F32, kind="ExternalInput")
    featd_in = nc.dram_tensor("featd", [DP, 64], BF16, kind="ExternalInput")
    selfz_in = nc.dram_tensor("selfz", [DP, 4], F32, kind="ExternalInput")
    iota_in = nc.dram_tensor("iota", [P, P], BF16, kind="ExternalInput")
    if l2:
        w2_in = nc.dram_tensor("w2", [P, 2, 64], BF16, kind="ExternalInput")
        ident_in = nc.dram_tensor("ident", [P, P], BF16, kind="ExternalInput")
    y_out = nc.dram_tensor("y", [DP, 64], F32, kind="ExternalOutput")

    # group schedule: runs of <= GC chunks, same half
    groups = []
    k = 0
    while k < nchunks:
        k1 = min(k + GC, nchunks, nlo if k < nlo else nchunks)
        groups.append((k, k1))
        k = k1
    first = [False] * nchunks
    last = [False] * nchunks
    wlast = [False] * nchunks
    seen = set()
    wl = {}
    for k, (w, h) in enumerate(chunks):
        if (h, w) not in seen:
            seen.add((h, w))
            first[k] = True
        if k + 1 >= nchunks or chunks[k + 1] != (w, h):
            last[k] = True
        wl[w] = k
    for w, k in wl.items():
        wlast[k] = True

    with tile.TileContext(nc) as tc:
        with (
            tc.tile_pool(name="c", bufs=1) as cp,
            tc.tile_pool(name="g", bufs=3 if l2 else 4) as gp,
            tc.tile_pool(name="r", bufs=2) as rp,
            tc.tile_pool(name="s", bufs=3) as sp,
            tc.tile_pool(name="pw", bufs=4, space="PSUM") as pw,
            tc.tile_pool(name="pt", bufs=2, space="PSUM") as pt,
        ):
            # big constant loads go on the Act engine's DGE so the first
            # gather groups' idx loads (sync engine) aren't queued behind them
            dloc_sb = cp.tile([P, nchunks], BF16)
            nc.scalar.dma_start(dloc_sb[:], dstloc[:])
            alde_full = cp.tile([P, nchunks, 4], F32)
            nc.scalar.dma_start(alde_full[:], alde_in[:])
            skipd_sb = cp.tile([P, NWIN, 64], F32)
            nc.scalar.dma_start(
                skipd_sb[:], skipd_in[:].rearrange("(i p) c -> p i c", p=P))
            featd_sb = cp.tile([P, NWIN, 64], BF16)
            nc.scalar.dma_start(
                featd_sb[:], featd_in[:].rearrange("(i p) c -> p i c", p=P))
            selfz_sb = cp.tile([P, NWIN, 4], F32)
            nc.scalar.dma_start(
                selfz_sb[:], selfz_in[:].rearrange("(i p) h -> p i h", p=P))
            iota_sb = cp.tile([P, 1, P], BF16)
            nc.scalar.dma_start(iota_sb[:, 0, :], iota_in[:])
            if l2:
                w2_sb = cp.tile([P, 2, 64], BF16)
                nc.scalar.dma_start(w2_sb[:], w2_in[:])
                ident_sb = cp.tile([P, P], BF16)
                nc.scalar.dma_start(ident_sb[:], ident_in[:])
            # bf16 msum for l2 frees ~25KB of SBUF for bigger gather groups;
            # only 3 rounded writes per value (init + 2 sweep drains)
            msum = cp.tile([P, NWIN, NW], BF16 if l2 else F32)
            y_sb = cp.tile([P, NWIN, 64], F32)

            # init msum with the dense self-loop contributions
            exself = cp.tile([P, NWIN, 4], F32)
            nc.vector.scalar_tensor_tensor(
                exself[:], selfz_sb[:], 0.2, selfz_sb[:],
                AluOpType.mult, AluOpType.max)
            nc.scalar.activation(exself[:], exself[:],
                                 mybir.ActivationFunctionType.Exp)
            nc.vector.tensor_copy(msum[:, :, NW - 4: NW], exself[:])
            if l2:
                nc.vector.tensor_tensor(
                    msum[:, :, 0:256].rearrange("p w (h c) -> p w h c", c=64),
                    featd_sb[:, :, None, :].to_broadcast([P, NWIN, 4, 64]),
                    exself[:, :, :, None].to_broadcast([P, NWIN, 4, 64]),
                    AluOpType.mult)
            else:
                nc.vector.tensor_tensor(
                    msum[:, :, 0:64].rearrange("p w (h c) -> p w h c", c=16),
                    featd_sb[:].rearrange("p w (h c) -> p w h c", c=16),
                    exself[:, :, :, None].to_broadcast([P, NWIN, 4, 16]),
                    AluOpType.mult)

            win_ps = {}
            for gi, (k0, k1) in enumerate(groups):
                T = k1 - k0
                h = chunks[k0][1]
                base = table[0: LO_ROWS + 1, :] if h == 0 \
                    else table[LO_ROWS + 1: TROWS, :]
                o16 = (k0 * P if h == 0 else (k0 - nlo) * P) // 16
                idx_t = sp.tile([P, GC * 8], I16, tag="idx")
                nc.sync.dma_start(
                    idx_t[:, : T * 8],
                    (idx_lo if h == 0 else idx_hi)[:, o16: o16 + T * 8])
                gt = gp.tile([P, GC, P], BF16, tag="g")
                nc.gpsimd.dma_gather(
                    gt[:, :T, :], base, idx_t[:, : T * 8], T * P, T * P, P,
                    single_packet=False, queue_num=gi % NQ)

                zf = sp.tile([P, GC, 4], F32, tag="z")
                gf = gt[:].bitcast(F32)
                nc.vector.tensor_tensor(zf[:, :T, :], gf[:, :T, 32:36],
                                        alde_full[:, k0:k1, :], AluOpType.add)
                nc.vector.scalar_tensor_tensor(
                    zf[:, :T, :], zf[:, :T, :], 0.2, zf[:, :T, :],
                    AluOpType.mult, AluOpType.max)
                sel = (rp if l2 else sp).tile([P, GC, P], BF16, tag="sel")
                nc.vector.tensor_tensor(
                    sel[:, :T, :],
                    iota_sb[:].to_broadcast([P, T, P]),
                    dloc_sb[:, k0:k1, None].to_broadcast([P, T, P]),
                    AluOpType.is_equal)

                if l2:
                    rhs = rp.tile([P, GC, 260], BF16, tag="rhs")
                    nc.scalar.activation(rhs[:, :T, 256:260], zf[:, :T, :],
                                         mybir.ActivationFunctionType.Exp)
                    nc.vector.tensor_tensor(
                        rhs[:, :T, 0:256].rearrange(
                            "p t (h c) -> p t h c", c=64),
                        gt[:, :T, None, 0:64].to_broadcast([P, T, 4, 64]),
                        rhs[:, :T, 256:260, None].to_broadcast([P, T, 4, 64]),
                        AluOpType.mult)
                else:
                    nc.scalar.activation(gt[:, :T, 64:68], zf[:, :T, :],
                                         mybir.ActivationFunctionType.Exp)
                    nc.vector.tensor_tensor(
                        gt[:, :T, 0:64].rearrange("p t (h c) -> p t h c", c=16),
                        gt[:, :T, 0:64].rearrange("p t (h c) -> p t h c", c=16),
                        gt[:, :T, 64:68, None].to_broadcast([P, T, 4, 16]),
                        AluOpType.mult)

                for t in range(T):
                    k = k0 + t
                    w, hh = chunks[k]
                    if first[k]:
                        win_ps[w] = pw.tile([P, NW], F32, space="PSUM",
                                            tag="win", name=f"win{w}h{hh}")
                    rhs_ap = rhs[:, t, :] if l2 else gt[:, t, 0:68]
                    nc.tensor.matmul(win_ps[w][:], sel[:, t, :], rhs_ap,
                                     start=first[k], stop=last[k],
                                     skip_group_check=True)
                    if last[k]:
                        pwin = win_ps.pop(w)
                        nc.vector.tensor_tensor(msum[:, w, :], msum[:, w, :],
                                                pwin[:], AluOpType.add)
                    if l2 and wlast[k]:
                        # per-window W2 drain, overlapped with later groups
                        recw = sp.tile([P, 4], F32, tag="recw")
                        nc.vector.reciprocal(recw[:], msum[:, w, 256:260])
                        snw = sp.tile([P, 4, 64], BF16, tag="snw")
                        nc.vector.tensor_tensor(
                            snw[:],
                            msum[:, w, 0:256].rearrange(
                                "p (h c) -> p h c", c=64),
                            recw[:, :, None].to_broadcast([P, 4, 64]),
                            AluOpType.mult)
                        yps = pt.tile([P, 64], F32, space="PSUM", tag="yps")
                        for j in range(2):
                            tp = pt.tile([P, P], BF16, space="PSUM", tag="tp")
                            nc.tensor.matmul(
                                tp[:],
                                snw[:].rearrange("p h c -> p (h c)")
                                      [:, j * P:(j + 1) * P],
                                ident_sb[:], is_transpose=True,
                                start=True, stop=True, skip_group_check=True)
                            st = sp.tile([P, P], BF16, tag="st")
                            nc.scalar.activation(
                                st[:], tp[:],
                                mybir.ActivationFunctionType.Copy)
                            nc.tensor.matmul(yps[:], st[:], w2_sb[:, j, :],
                                             start=(j == 0), stop=(j == 1),
                                             skip_group_check=True)
                        nc.vector.scalar_tensor_tensor(
                            y_sb[:, w, :], yps[:], 0.25, skipd_sb[:, w, :],
                            AluOpType.mult, AluOpType.add)

            if not l2:
                rec = cp.tile([P, NWIN, 4], F32)
                nc.vector.reciprocal(rec[:], msum[:, :, 64:68])
                nc.vector.tensor_tensor(
                    y_sb[:].rearrange("p w (h c) -> p w h c", c=16),
                    msum[:, :, 0:64].rearrange("p w (h c) -> p w h c", c=16),
                    rec[:, :, :, None].to_broadcast([P, NWIN, 4, 16]),
                    AluOpType.mult)
                nc.vector.tensor_tensor(y_sb[:], y_sb[:], skipd_sb[:],
                                        AluOpType.add)
                nc.vector.tensor_scalar_max(y_sb[:], y_sb[:], 0.0)
            nc.sync.dma_start(
                y_out[:].rearrange("(i p) c -> p i c", p=P), y_sb[:])
    nc.compile()
    return nc


# ------------------------------------------------------------------ driver

_CACHE = {}
_DBG = []
_EXEC_NS = []
_RESULTS = []


def _blockdiag(a):
    H, C = a.shape
    m = np.zeros((H * C, H), np.float32)
    for hh in range(H):
        m[hh * C: (hh + 1) * C, hh] = a[hh]
    return m


def _bf(x):
    return np.ascontiguousarray(np.asarray(x, np.float32)
                                .astype(ml_dtypes.bfloat16))


def kernel(**inp):
    x = np.asarray(inp["x"], np.float32)
    ei = np.asarray(inp["edge_index"], np.int64)
    N, IN = x.shape
    E = ei.shape[1]

    # self-loops are handled densely in launch B; streams carry real edges
    src = ei[0]
    dst = ei[1]

    pkey = ("plan", N, E, hash(ei.tobytes()))
    if pkey not in _CACHE:
        _CACHE[pkey] = build_plan(src, dst, N)
    shared, plans = _CACHE[pkey]

    def prep01(Wv, a_s, a_d, cb, sW, sb, g, b, m, v):
        Wv, sW = np.asarray(Wv, np.float32), np.asarray(sW, np.float32)
        bns = (np.asarray(g) / np.sqrt(np.asarray(v) + EPS)).astype(np.float32)
        bnt = (np.asarray(b) - np.asarray(m) * bns).astype(np.float32)
        Waug = np.concatenate(
            [Wv * bns[None, :], Wv @ _blockdiag(np.asarray(a_s)),
             Wv @ _blockdiag(np.asarray(a_d))], 1)
        return (Waug, sW * bns[None, :],
                (np.asarray(cb) + np.asarray(sb)) * bns + bnt, None)

    def prep2(Wv, a_s, a_d, cb, sW, sb):
        Wv = np.asarray(Wv, np.float32)
        Waug = np.concatenate(
            [np.eye(64, dtype=np.float32), Wv @ _blockdiag(np.asarray(a_s)),
             Wv @ _blockdiag(np.asarray(a_d))], 1)
        w2 = np.ascontiguousarray(
            Wv.reshape(64, 4, 64).transpose(1, 0, 2).reshape(256, 64)
            .reshape(2, 128, 64).transpose(1, 0, 2))
        return (Waug, np.asarray(sW, np.float32),
                np.asarray(cb) + np.asarray(sb), w2)

    Ls = [
        prep01(inp["conv0_W"], inp["conv0_as"], inp["conv0_ad"],
               inp["conv0_b"], inp["skip0_W"], inp["skip0_b"],
               inp["bn0_g"], inp["bn0_b"], inp["bn0_m"], inp["bn0_v"]),
        prep01(inp["conv1_W"], inp["conv1_as"], inp["conv1_ad"],
               inp["conv1_b"], inp["skip1_W"], inp["skip1_b"],
               inp["bn1_g"], inp["bn1_b"], inp["bn1_m"], inp["bn1_v"]),
        prep2(inp["conv2_W"], inp["conv2_as"], inp["conv2_ad"],
              inp["conv2_b"], inp["skip2_W"], inp["skip2_b"]),
    ]

    iota_np = np.tile(np.arange(P, dtype=np.float32), (P, 1)).astype(
        ml_dtypes.bfloat16)
    ident_np = np.eye(P, dtype=np.float32).astype(ml_dtypes.bfloat16)
    # sentinel row: zero feats, al_src = -40 (f32 packed in bf16 slots 64..71)
    sent_view = np.zeros(P, np.uint16)
    sent_view[64:72] = np.full(4, SENT_ALS, np.float32).view(np.uint16)
    sent = sent_view.view(ml_dtypes.bfloat16)

    h = x
    for li in range(3):
        F = IN if li == 0 else 64
        l2 = li == 2
        Waug, skipWf, biasv, w2 = Ls[li]
        akey = ("A", F)
        if akey not in _CACHE:
            _CACHE[akey] = build_dense(F)
        bkey = ("B", l2, pkey)  # schedule is baked into the B kernel
        if bkey not in _CACHE:
            _CACHE[bkey] = build_edge(shared, l2)

        hT = h.T.astype(np.float32)
        base_a = {
            "Waug": _bf(Waug),
            "skipW": _bf(skipWf),
            "biasR": np.tile(np.asarray(biasv, np.float32), (P, 1)),
        }
        a_maps = []
        for c in range(NC):
            node = c * SB + PERMC
            valid_s = PERMC < SB
            hts = np.zeros((F, DP), np.float32)
            hts[:, valid_s] = hT[:, node[valid_s]]
            vm = plans[c]["vmap"][PERMC]
            valid_d = vm >= 0
            htow = np.zeros((F, DP), np.float32)
            htow[:, valid_d] = hT[:, vm[valid_d]]
            a_maps.append(dict(base_a, hTs=_bf(hts), hTow=_bf(htow)))
        res_a = run_bass_kernel_spmd(_CACHE[akey], a_maps,
                                     core_ids=list(range(NC)))
        _RESULTS.append(res_a)
        if res_a.exec_time_ns:
            _EXEC_NS.append(res_a.exec_time_ns)

        tbl = np.empty((TROWS, P), ml_dtypes.bfloat16)
        for c in range(4):
            tbl[c * RB:(c + 1) * RB] = res_a.results[c]["tshard"]
        tbl[LO_ROWS] = sent
        for c in range(4, 8):
            tbl[LO_ROWS + 1 + (c - 4) * RB: LO_ROWS + 1 + (c - 3) * RB] = \
                res_a.results[c]["tshard"]
        tbl[TROWS - 1] = sent

        base_b = {"table": tbl, "iota": iota_np}
        if l2:
            base_b["w2"] = _bf(w2)
            base_b["ident"] = ident_np
        b_maps = []
        for c in range(NC):
            aldv = res_a.results[c]["aldv"]
            alde = aldv[plans[c]["slot_dst"]]  # [128, nchunks, 4]
            b_maps.append(dict(
                base_b,
                idx_lo=plans[c]["idx_lo"], idx_hi=plans[c]["idx_hi"],
                dstloc=plans[c]["dstloc"],
                alde=np.ascontiguousarray(alde.astype(np.float32)),
                skipd=np.ascontiguousarray(
                    res_a.results[c]["skipd"].astype(np.float32)),
                featd=np.ascontiguousarray(res_a.results[c]["featd"]),
                selfz=np.ascontiguousarray(
                    res_a.results[c]["selfz"].astype(np.float32))))
        res_b = run_bass_kernel_spmd(_CACHE[bkey], b_maps,
                                     core_ids=list(range(NC)))
        _RESULTS.append(res_b)
        if res_b.exec_time_ns:
            _EXEC_NS.append(res_b.exec_time_ns)

        hn = np.zeros((N, 64), np.float32)
        for c in range(NC):
            vm = plans[c]["vmap"]
            valid = vm >= 0
            hn[vm[valid]] = res_b.results[c]["y"][valid]
        h = hn
        _DBG.append(h)
    return h
